# revision 51
# baseline (speedup 1.0000x reference)
"""Trainium2 Bass kernel for nn_H_ATT (GatedTrans pair-attention block).

Math (per example):
  HE = tanh(hist@W_hy+b_hy) * lrelu(hist@W_hg+b_hg)      [R, H]
  QE = tanh(ques@W_qy+b_qy) * lrelu(ques@W_qg+b_qg)      [R, H]
  num[q,h]  = sum_k QE[q,k]*W_att[k]*HE[h,k]
  den[q,h]  = sqrt(sum_k QE[q,k]^2 * HE[h,k]^2)
  s = num / max(den, eps)          (b_att cancels in softmax)
  att = causal_softmax(s)          (softmax*tril/renorm == masked softmax)
  feat = att @ hist                 [R, 2H]

Sharding: pure data parallel, 8 examples per core on 8 NeuronCores.

Shipping mode "ws8" (weight-stationary, ~64us vs the 91us dp8dr baseline):
  - fp8e4 DoubleRow GEMMs with the WEIGHT tile stationary and the
    transposed activations moving: outputs land directly in [h-dim, rows]
    orientation, so no PE transposes and no per-tile evacuation storm.
  - lrelu via Prelu(alpha=0.01) on the Scalar engine; scores use an
    ln/exp-based rsqrt so the tail needs a single activation-table switch.
  - W_att folds into the hist-side evacuation (per-partition there);
    softmax normalization folds into the feat-PSUM evacuation.
  - DMA: the two HWDGE rings generate ~one descriptor per 45ns (128 descs
    per transfer, one per partition), so they carry only the
    first-needed pieces; the bulk weight stream rides SWDGE (gpsimd),
    which emits serially (~2.8us/dma) but sustains wire rate.
  - Dependency-free warmup matmuls keep the PE's DVFS ramp warm during
    the initial weight-DMA window (idle gaps halve the matmul rate for
    ~3.4us afterwards).
The big-GEMM operands are scaled on host (weights x1024, activations x16);
tanh descales via the activation `scale` param, the lrelu branch stays
scaled (positively homogeneous -> cancels in the num/den score ratio).
"""

import numpy as np
import ml_dtypes

import bass_rust
import concourse.bass as bass
import concourse.mybir as mybir
import concourse.tile as tile
from concourse.vector_clock import ScopedClock

# ---------------------------------------------------------------------------
# Workaround: this walrus build accepts only ONE semaphore wait on an SP
# Drain, but TileContext's tail drain carries one wait per live semaphore.
# Split them across a chain of drains.
# ---------------------------------------------------------------------------


def _patched_drain_and_barrier(self, tick_clock, wait_clock):
    nc = self.nc
    drain_inst = nc.sync.drain()
    wait_clock.add_sem_waits(
        drain_inst.ins, ScopedClock({None: tick_clock.global_clock})
    )
    waits = list(drain_inst.ins.sync_info.on_wait)
    if len(waits) > 1:
        drain_inst.ins.sync_info = bass_rust.SyncInfo(
            on_wait=waits[:1], on_update=list(drain_inst.ins.sync_info.on_update)
        )
        for i in range(1, len(waits)):
            extra = nc.sync.drain()
            extra.ins.sync_info = bass_rust.SyncInfo(
                on_wait=waits[i : i + 1], on_update=[]
            )
    nc.all_engine_barrier()
    assert self.sems is not None
    popped = nc._tile_sem_poison_stack.pop()
    assert popped is self._sem_poison
    nc.clear_and_free_semaphores(list(self.sems.allocated().values()))
    nc.all_engine_barrier()


tile.TileContext._drain_and_barrier = _patched_drain_and_barrier


def _split_multi_waits(nc):
    """This walrus build accepts at most one semaphore wait per instruction.
    Hoist extra waits onto standalone EventSemaphore instructions inserted
    just before the owning instruction in the same engine's stream."""
    uid = [0]
    for f in nc.m.functions:
        for bb in f.blocks:
            out = []
            for inst in bb.instructions:
                si = inst.sync_info
                if si is not None and len(si.on_wait) > 1:
                    waits = list(si.on_wait)
                    for w in waits[:-1]:
                        nop = mybir.InstEventSemaphore(
                            name=f"I-waitsplit-{uid[0]}", ins=[], outs=[]
                        )
                        uid[0] += 1
                        nop.engine = inst.engine
                        nop.sync_info = bass_rust.SyncInfo(
                            on_wait=[w], on_update=[]
                        )
                        out.append(nop)
                    inst.sync_info = bass_rust.SyncInfo(
                        on_wait=[waits[-1]], on_update=list(si.on_update)
                    )
                out.append(inst)
            bb.instructions[:] = out

# ---------------------------------------------------------------------------

B, R, H, IN = 64, 32, 1024, 2048
NCORES = 8
BL = B // NCORES  # examples per core
BR = BL * R  # 256 rows per core
KC = IN // 128  # 16 contraction chunks
MC = H // 128  # 8 h chunks
NEG = -1.0e30
S_X = 16.0  # fp8 activation scale
S_W = 1024.0  # fp8 weight scale

F32 = mybir.dt.float32
BF16 = mybir.dt.bfloat16

_XDT = {
    "f32r": mybir.dt.float32r,
    "bf16": mybir.dt.bfloat16,
    "fp8": mybir.dt.float8e4,
}
_XNP = {
    "f32r": np.float32,
    "bf16": ml_dtypes.bfloat16,
    "fp8": ml_dtypes.float8_e4m3,
}


def build_program(mode="fp8", zero_bias=True):
    """Per-core Bass program. mode in {"f32r", "bf16", "fp8"} selects the
    dtype of the big-GEMM operands (weights + transposed activations)."""
    xdt = _XDT[mode]
    FEAT_DT = BF16
    # fp8 inputs are pre-scaled: psum = (S_X*x)@(S_W*w)
    descale = 1.0 / (S_X * S_W) if mode == "fp8" else 1.0

    nc = bass.Bass()
    qt_d = nc.dram_tensor("qt", [128, KC, BR], xdt, kind="ExternalInput")
    ht_d = nc.dram_tensor("ht", [128, KC, BR], xdt, kind="ExternalInput")
    hn_d = nc.dram_tensor("hn", [128, 2, IN], FEAT_DT, kind="ExternalInput")
    wh_d = nc.dram_tensor("wh", [MC, 128, 2, KC, 128], xdt, kind="ExternalInput")
    wq_d = nc.dram_tensor("wq", [MC, 128, 2, KC, 128], xdt, kind="ExternalInput")
    b_d = {
        n: nc.dram_tensor(n, [128, MC], F32, kind="ExternalInput")
        for n in ("bhy", "bhg", "bqy", "bqg")
    }
    watt_d = nc.dram_tensor("watt", [128, MC], F32, kind="ExternalInput")
    mask_d = nc.dram_tensor("mask", [128, 128], F32, kind="ExternalInput")
    ident_d = nc.dram_tensor("ident", [128, 128], F32, kind="ExternalInput")
    feat_d = nc.dram_tensor("feat", [2, 128, IN], FEAT_DT, kind="ExternalOutput")

    ACT = mybir.ActivationFunctionType

    with tile.TileContext(nc) as tc:
        with (
            tc.tile_pool(name="big", bufs=1) as big,
            tc.tile_pool(name="wts", bufs=5) as wts,
            tc.tile_pool(name="tmp", bufs=3) as tmp,
            tc.tile_pool(name="sm", bufs=1) as sm,
        ):
            # ques-transposed activations: needed first
            qt = big.tile([128, KC, BR], xdt, tag="qt")
            for q4 in range(4):
                ks = slice(4 * q4, 4 * (q4 + 1))
                nc.sync.dma_start(qt[:, ks, :], qt_d[:, ks, :])

            EDT = mybir.dt.bfloat16
            he = big.tile([128, MC, BR], EDT, tag="he")
            he2 = big.tile([128, MC, BR], EDT, tag="he2")
            qew = big.tile([128, MC, BR], EDT, tag="qew")
            qe2 = big.tile([128, MC, BR], EDT, tag="qe2")

            with (
                tc.tile_pool(name="pse", bufs=3, space="PSUM") as pse,
                tc.tile_pool(name="psnd", bufs=1, space="PSUM") as psnd,
            ):
                num_ps = [psnd.tile([128, 128], F32, name=f"num{g}", tag=f"num{g}") for g in range(2)]
                den_ps = [psnd.tile([128, 128], F32, name=f"den{g}", tag=f"den{g}") for g in range(2)]

                def gated(xt, w_dram, by, bg, m):
                    """One fused y+g weight DMA; returns (ty, tg) [128, BR].
                    ty is descaled; tg keeps the fp8 input scale (cancels in
                    the num/den ratio when biases are zero)."""
                    wt = wts.tile([128, 2, KC, 128], xdt, tag="wt")
                    nc.sync.dma_start(wt[:], w_dram[m])
                    psy = pse.tile([128, BR], F32, tag="psy")
                    for k in range(KC):
                        nc.tensor.matmul(
                            psy[:], wt[:, 0, k, :], xt[:, k, :],
                            start=(k == 0), stop=(k == KC - 1),
                        )
                    psg = pse.tile([128, BR], F32, tag="psg")
                    for k in range(KC):
                        nc.tensor.matmul(
                            psg[:], wt[:, 1, k, :], xt[:, k, :],
                            start=(k == 0), stop=(k == KC - 1),
                        )
                    ty = tmp.tile([128, BR], F32, tag="ty")
                    nc.scalar.activation(
                        ty[:], psy[:], ACT.Tanh, bias=by[:, m : m + 1],
                        scale=descale,
                    )
                    t1 = tmp.tile([128, BR], F32, tag="t1")
                    tg = tmp.tile([128, BR], F32, tag="tg")
                    if zero_bias:
                        # leaky_relu(x) = max(x, 0.01x); input scale cancels
                        nc.vector.tensor_scalar_mul(t1[:], psg[:], 0.01)
                        nc.vector.tensor_max(tg[:], psg[:], t1[:])
                    else:
                        # descale then leaky_relu(x+b) = max(x+b, 0.01*(x+b))
                        if mode == "fp8":
                            nc.scalar.activation(
                                psg[:], psg[:], ACT.Identity, scale=descale
                            )
                        nc.vector.tensor_scalar(
                            t1[:], psg[:], bg[:, m : m + 1], 0.01,
                            op0=mybir.AluOpType.add, op1=mybir.AluOpType.mult,
                        )
                        nc.vector.tensor_scalar_add(tg[:], psg[:], bg[:, m : m + 1])
                        nc.vector.tensor_max(tg[:], tg[:], t1[:])
                    return ty, tg

                # consts land while the first ques matmuls run
                bsb = {}
                for n in ("bqy", "bqg", "bhy", "bhg"):
                    bsb[n] = sm.tile([128, MC], F32, name=n, tag=n)
                    nc.sync.dma_start(bsb[n][:], b_d[n][:])
                watt = sm.tile([128, MC], F32, tag="watt")
                nc.sync.dma_start(watt[:], watt_d[:])

                # ques embeddings (first: only needs qt + wq)
                for m in range(MC):
                    ty, tg = gated(qt, wq_d, bsb["bqy"], bsb["bqg"], m)
                    nc.vector.scalar_tensor_tensor(
                        qew[:, m, :], ty[:], watt[:, m : m + 1], tg[:],
                        op0=mybir.AluOpType.mult, op1=mybir.AluOpType.mult,
                    )
                    qe = tmp.tile([128, BR], F32, tag="qe")
                    nc.vector.tensor_mul(qe[:], ty[:], tg[:])
                    nc.scalar.square(qe2[:, m, :], qe[:])
                    if m == 0:
                        # hist-transposed activations: stream in during ques phase
                        ht = big.tile([128, KC, BR], xdt, tag="ht")
                        nc.sync.dma_start(ht[:], ht_d[:])

                # hist embeddings + num/den accumulation per chunk
                for m in range(MC):
                    ty, tg = gated(ht, wh_d, bsb["bhy"], bsb["bhg"], m)
                    nc.vector.tensor_mul(he[:, m, :], ty[:], tg[:])
                    nc.scalar.square(he2[:, m, :], he[:, m, :])
                    for g in range(2):
                        sl = slice(128 * g, 128 * (g + 1))
                        nc.tensor.matmul(
                            num_ps[g][:], qew[:, m, sl], he[:, m, sl],
                            start=(m == 0), stop=(m == MC - 1),
                        )
                        nc.tensor.matmul(
                            den_ps[g][:], qe2[:, m, sl], he2[:, m, sl],
                            start=(m == 0), stop=(m == MC - 1),
                        )
                    if m == 0:
                        # feat inputs: stream in during hist phase
                        hn = big.tile([128, 2, IN], FEAT_DT, tag="hn")
                        nc.sync.dma_start(hn[:], hn_d[:])
                        mask = sm.tile([128, 128], F32, tag="mask")
                        nc.sync.dma_start(mask[:], mask_d[:])
                        ident = sm.tile([128, 128], F32, tag="ident")
                        nc.sync.dma_start(ident[:], ident_d[:])

                # scores while num/den PSUM is still available
                sc = []
                for g in range(2):
                    sd = tmp.tile([128, 128], F32, tag="sd")
                    nc.scalar.activation(sd[:], den_ps[g][:], ACT.Sqrt)
                    rd = tmp.tile([128, 128], F32, tag="rd")
                    nc.vector.reciprocal(rd[:], sd[:])
                    s = sm.tile([128, 128], F32, name=f"sc{g}", tag=f"sc{g}")
                    nc.vector.tensor_mul(s[:], num_ps[g][:], rd[:])
                    nc.vector.tensor_add(s[:], s[:], mask[:])
                    sc.append(s)

            # attention tail + feat
            with (
                tc.tile_pool(name="psa", bufs=1, space="PSUM") as psa,
                tc.tile_pool(name="psf", bufs=4, space="PSUM") as psf,
            ):
                for g in range(2):
                    s = sc[g]
                    att = sm.tile([128, 128], F32, name=f"att{g}", tag=f"att{g}")
                    rs = sm.tile([128, 1], F32, name=f"rs{g}", tag=f"rs{g}")
                    # off-diagonal blocks are hard-masked to NEG, so one
                    # full-width exp works and accumulates the row sums
                    nc.scalar.activation(
                        att[:], s[:], ACT.Exp, accum_out=rs[:]
                    )
                    rrs = sm.tile([128, 1], F32, name=f"rrs{g}", tag=f"rrs{g}")
                    nc.vector.reciprocal(rrs[:], rs[:])
                    nc.vector.tensor_scalar_mul(att[:], att[:], rrs[:])
                    atp = psa.tile([128, 128], F32, tag="atp")
                    nc.tensor.transpose(atp[:], att[:], ident[:])
                    atb = sm.tile([128, 128], FEAT_DT, name=f"atb{g}", tag=f"atb{g}")
                    nc.scalar.copy(atb[:], atp[:])
                    for c2 in range(2):
                        fsb = tmp.tile([128, 1024], FEAT_DT, tag="fsb")
                        for half in range(2):
                            c = 2 * c2 + half
                            cs = slice(512 * c, 512 * (c + 1))
                            fps = psf.tile([128, 512], F32, tag="fps")
                            nc.tensor.matmul(
                                fps[:], atb[:], hn[:, g, cs], start=True, stop=True
                            )
                            dst = fsb[:, 512 * half : 512 * (half + 1)]
                            if half == 0:
                                nc.scalar.copy(dst, fps[:])
                            else:
                                nc.vector.tensor_copy(dst, fps[:])
                        nc.sync.dma_start(
                            feat_d[g, :, 1024 * c2 : 1024 * (c2 + 1)], fsb[:]
                        )

    _split_multi_waits(nc)
    return nc


# ---------------------------------------------------------------------------
# Data-parallel DoubleRow program ("dp8dr"): fp8 DoubleRow big GEMMs with the
# ACTIVATION row-block stationary (so one 256-col LDWEIGHTS serves 4 N=512
# matmuls and the 2x fp8 rate is realized), weights as the moving operand.
# The GEMM outputs land [rows, H]; PE transposes per 128-h-chunk restore the
# [k, rows] orientation the num/den contraction needs, with W_att / square
# fused into the transpose-PSUM evacuation. Tail identical to the fp8 mode.
# DMAs are split across the two HWDGE rings (SP: ques side, ACT: hist side).
# ---------------------------------------------------------------------------


def build_program_dp(zero_bias=True, debug=False):
    nc = bass.Bass()
    DR = mybir.MatmulPerfMode.DoubleRow
    descale = 1.0 / (S_X * S_W)
    KC2L = IN // 256  # 8

    qt_d = nc.dram_tensor("qt", [128, KC2L, 2, BR], FP8, kind="ExternalInput")
    ht_d = nc.dram_tensor("ht", [128, KC2L, 2, BR], FP8, kind="ExternalInput")
    # moving weights: [input][128p][k2][j][branch][H]
    wq_d = nc.dram_tensor("wq", [128, KC2L, 2, 2, H], FP8, kind="ExternalInput")
    wh_d = nc.dram_tensor("wh", [128, KC2L, 2, 2, H], FP8, kind="ExternalInput")
    hn_d = nc.dram_tensor("hn", [128, 2, IN], BF16, kind="ExternalInput")
    b_d = {
        n: nc.dram_tensor(n, [128, MC], F32, kind="ExternalInput")
        for n in ("bhy", "bhg", "bqy", "bqg")
    }
    watt_d = nc.dram_tensor("watt", [128, MC], F32, kind="ExternalInput")
    mask_d = nc.dram_tensor("mask", [128, 128], F32, kind="ExternalInput")
    ident_d = nc.dram_tensor("ident", [128, 128], F32, kind="ExternalInput")
    feat_d = nc.dram_tensor("feat", [2, 128, IN], BF16, kind="ExternalOutput")
    dbg_d = (
        nc.dram_tensor("dbg", [128, 4, MC, BR], BF16, kind="ExternalOutput")
        if debug
        else None
    )
    dbg2_d = (
        nc.dram_tensor("dbg2", [128, 4, 128], F32, kind="ExternalOutput")
        if debug
        else None
    )

    ACT = mybir.ActivationFunctionType

    with tile.TileContext(nc) as tc:
        with (
            tc.tile_pool(name="big", bufs=1) as big,
            tc.tile_pool(name="wts", bufs=2) as wts,
            tc.tile_pool(name="tmp", bufs=3) as tmp,
            tc.tile_pool(name="sm", bufs=1) as sm,
        ):
            # activations first (small), then weight chunks finely interleaved
            # over BOTH HWDGE rings (SP + ACT) so the GEMM is never DMA-gated
            qt = big.tile([128, KC2L, 2, BR], FP8, tag="qt")
            nc.sync.dma_start(qt[:, 0:4], qt_d[:, 0:4])
            nc.sync.dma_start(qt[:, 4:8], qt_d[:, 4:8])
            watt = sm.tile([128, MC], F32, tag="watt")
            nc.scalar.dma_start(watt[:], watt_d[:])
            mask = sm.tile([128, 128], F32, tag="mask")
            nc.scalar.dma_start(mask[:], mask_d[:])
            ident = sm.tile([128, 128], F32, tag="ident")
            nc.scalar.dma_start(ident[:], ident_d[:])
            wqf = big.tile([128, KC2L, 2, 2, H], FP8, tag="wqf")
            whf = big.tile([128, KC2L, 2, 2, H], FP8, tag="whf")
            ht = big.tile([128, KC2L, 2, BR], FP8, tag="ht")
            for k2 in range(KC2L):
                for j in range(2):
                    eng = nc.sync if (2 * k2 + j) % 2 == 0 else nc.scalar
                    eng.dma_start(wqf[:, k2, j], wq_d[:, k2, j])
            nc.scalar.dma_start(ht[:, 0:4], ht_d[:, 0:4])
            nc.sync.dma_start(ht[:, 4:8], ht_d[:, 4:8])
            for k2 in range(KC2L):
                for j in range(2):
                    eng = nc.sync if (2 * k2 + j) % 2 == 0 else nc.scalar
                    eng.dma_start(whf[:, k2, j], wh_d[:, k2, j])

            EDT = mybir.dt.bfloat16
            he = big.tile([128, MC, BR], EDT, tag="he")
            he2 = big.tile([128, MC, BR], EDT, tag="he2")
            qew = big.tile([128, MC, BR], EDT, tag="qew")
            qe2 = big.tile([128, MC, BR], EDT, tag="qe2")

            with (
                tc.tile_pool(name="psg", bufs=1, space="PSUM") as psg,
                tc.tile_pool(name="pst", bufs=2, space="PSUM") as pst,
                tc.tile_pool(name="psnd", bufs=1, space="PSUM") as psnd,
            ):
                gb = [
                    psg.tile([128, 512], F32, name=f"gb{j}", tag=f"gb{j}")
                    for j in range(4)  # y0 y1 g0 g1
                ]
                ndps = [
                    psnd.tile([128, 128], F32, name=f"nd{g}", tag=f"nd{g}")
                    for g in range(2)
                ]

                assert zero_bias, "dp8dr supports zero biases only"

                def emit_mms(i, rb):
                    xt = (qt, ht)[i]
                    wf = (wqf, whf)[i]
                    rsl = slice(128 * rb, 128 * (rb + 1))
                    for k2 in range(KC2L):
                        for b in range(2):
                            for hh in range(2):
                                nc.tensor.matmul(
                                    gb[2 * b + hh][:],
                                    xt[:, k2, :, rsl],
                                    wf[:, k2, :, b, 512 * hh : 512 * (hh + 1)],
                                    start=(k2 == 0),
                                    stop=(k2 == KC2L - 1),
                                    perf_mode=DR,
                                )

                def emit_evac(i, rb):
                    # ty=tanh(y), tg=lrelu(g), qe=ty*tg  [rows, H]
                    tyf = tmp.tile([128, H], F32, tag="tyf")
                    tgf = tmp.tile([128, H], F32, tag="tgf")
                    qe = tmp.tile([128, H], F32, tag="qe")
                    for hh in range(2):
                        hsl = slice(512 * hh, 512 * (hh + 1))
                        nc.scalar.activation(
                            tyf[:, hsl], gb[hh][:], ACT.Tanh, scale=descale
                        )
                        ps = gb[2 + hh]
                        t1 = tmp.tile([128, 512], F32, tag="t1")
                        nc.vector.tensor_scalar_mul(t1[:], ps[:], 0.01)
                        nc.vector.tensor_max(tgf[:, hsl], ps[:], t1[:])
                        nc.vector.tensor_mul(
                            qe[:, hsl], tyf[:, hsl], tgf[:, hsl]
                        )
                    return qe

                def emit_transposes(i, rb, qe, ms):
                    # [rows, H] -> [k, rows] per 128-h-chunk; fold watt/square
                    rsl = slice(128 * rb, 128 * (rb + 1))
                    for m in ms:
                        csl = slice(128 * m, 128 * (m + 1))
                        tp = pst.tile([128, 128], F32, tag="tp")
                        nc.tensor.transpose(tp[:], qe[:, csl], ident[:])
                        if i == 0:
                            qes = tmp.tile([128, 128], BF16, tag="qes")
                            nc.vector.tensor_copy(qes[:], tp[:])
                            nc.vector.tensor_scalar_mul(
                                qew[:, m, rsl], tp[:], watt[:, m : m + 1]
                            )
                            nc.vector.tensor_mul(qe2[:, m, rsl], tp[:], qes[:])
                        else:
                            nc.vector.tensor_copy(he[:, m, rsl], tp[:])
                            nc.vector.tensor_mul(
                                he2[:, m, rsl], tp[:], he[:, m, rsl]
                            )

                blocks = [(0, 0), (0, 1), (1, 0), (1, 1)]
                pending = None
                for blk in blocks:
                    emit_mms(*blk)
                    qe = emit_evac(*blk)
                    if pending is not None:
                        emit_transposes(*pending, range(MC))
                    pending = (*blk, qe)
                last_i, last_rb, last_qe = pending

                # preload the sqrt table set (square stays available in it)
                dummy = tmp.tile([128, 1], F32, tag="dummy")
                nc.scalar.activation(dummy[:], watt[:, 0:1], ACT.Sqrt)

                # final block's transposes, then num (evacuated to SBUF)
                # and den accumulation reusing the same banks: a start=True
                # wipes the WHOLE bank, so num and den cannot share one.
                emit_transposes(last_i, last_rb, last_qe, range(MC))
                hn = big.tile([128, 2, IN], BF16, tag="hn")
                nc.scalar.dma_start(hn[:], hn_d[:])
                nsb = [
                    sm.tile([128, 128], F32, name=f"nsb{g}", tag=f"nsb{g}")
                    for g in range(2)
                ]
                for m in range(MC):
                    for g in range(2):
                        sl = slice(128 * g, 128 * (g + 1))
                        nc.tensor.matmul(
                            ndps[g][:], qew[:, m, sl], he[:, m, sl],
                            start=(m == 0), stop=(m == MC - 1),
                        )
                for g in range(2):
                    nc.vector.tensor_copy(nsb[g][:], ndps[g][:])
                for m in range(MC):
                    for g in range(2):
                        sl = slice(128 * g, 128 * (g + 1))
                        nc.tensor.matmul(
                            ndps[g][:], qe2[:, m, sl], he2[:, m, sl],
                            start=(m == 0), stop=(m == MC - 1),
                        )

                if dbg_d is not None:
                    for t, ten in enumerate((qew, qe2, he, he2)):
                        nc.sync.dma_start(dbg_d[:, t], ten[:])

                # scores: s = num / sqrt(den2) + mask
                sc = []
                for g in range(2):
                    sd = tmp.tile([128, 128], F32, tag="sd")
                    nc.scalar.activation(sd[:], ndps[g][:], ACT.Sqrt)
                    if g == 1:
                        # start the exp table load while the DVE chain runs
                        nc.scalar.activation(dummy[:], watt[:, 0:1], ACT.Exp)
                    rd = tmp.tile([128, 128], F32, tag="rd")
                    nc.vector.reciprocal(rd[:], sd[:])
                    s = sm.tile([128, 128], F32, name=f"sc{g}", tag=f"sc{g}")
                    nc.vector.tensor_mul(s[:], nsb[g][:], rd[:])
                    nc.vector.tensor_add(s[:], s[:], mask[:])
                    sc.append(s)

            with (
                tc.tile_pool(name="psa", bufs=1, space="PSUM") as psa,
                tc.tile_pool(name="psf", bufs=4, space="PSUM") as psf,
            ):
                for g in range(2):
                    s = sc[g]
                    att = sm.tile([128, 128], F32, name=f"att{g}", tag=f"att{g}")
                    rs = sm.tile([128, 1], F32, name=f"rs{g}", tag=f"rs{g}")
                    nc.scalar.activation(att[:], s[:], ACT.Exp, accum_out=rs[:])
                    rrs = sm.tile([128, 1], F32, name=f"rrs{g}", tag=f"rrs{g}")
                    nc.vector.reciprocal(rrs[:], rs[:])
                    nc.vector.tensor_scalar_mul(att[:], att[:], rrs[:])
                    atp = psa.tile([128, 128], F32, tag="atp")
                    nc.tensor.transpose(atp[:], att[:], ident[:])
                    atb = sm.tile([128, 128], BF16, name=f"atb{g}", tag=f"atb{g}")
                    nc.scalar.copy(atb[:], atp[:])
                    if dbg2_d is not None:
                        nc.sync.dma_start(dbg2_d[:, g], s[:])
                        nc.sync.dma_start(dbg2_d[:, 2 + g], att[:])
                    for c2 in range(2):
                        fsb = tmp.tile([128, 1024], BF16, tag="fsb")
                        for half in range(2):
                            c = 2 * c2 + half
                            cs = slice(512 * c, 512 * (c + 1))
                            fps = psf.tile([128, 512], F32, tag="fps")
                            nc.tensor.matmul(
                                fps[:], atb[:], hn[:, g, cs], start=True, stop=True
                            )
                            dst = fsb[:, 512 * half : 512 * (half + 1)]
                            nc.vector.tensor_copy(dst, fps[:])
                        nc.sync.dma_start(
                            feat_d[g, :, 1024 * c2 : 1024 * (c2 + 1)], fsb[:]
                        )

    _split_multi_waits(nc)
    return nc


def _prep_dp(hist, ques, W_hy, b_hy, W_hg, b_hg, W_qy, b_qy, W_qg, b_qg, W_att):
    E4 = ml_dtypes.float8_e4m3
    KC2L = IN // 256

    def wpack(Wy, Wg):
        # [IN, H] x2 -> [128, KC2, 2, 2, H]; w[p,k2,j,b,h] = S_W*W_b[256k2+128j+p, h]
        a = np.stack(
            [
                np.clip(Wy * S_W, -240, 240),
                np.clip(Wg * S_W, -240, 240),
            ],
            axis=1,
        )  # [IN, 2, H]
        return np.ascontiguousarray(
            a.reshape(KC2L, 2, 128, 2, H).transpose(2, 0, 1, 3, 4)
        ).astype(E4)

    def bvec(b):
        return np.ascontiguousarray(b.reshape(MC, 128).T).astype(np.float32)

    m32 = np.where(
        np.arange(32)[None, :] <= np.arange(32)[:, None], 0.0, NEG
    ).astype(np.float32)
    mask = np.full((128, 128), NEG, np.float32)
    for e in range(4):
        mask[32 * e : 32 * e + 32, 32 * e : 32 * e + 32] = m32

    shared = {
        "wq": wpack(W_qy, W_qg),
        "wh": wpack(W_hy, W_hg),
        "bhy": bvec(b_hy),
        "bhg": bvec(b_hg),
        "bqy": bvec(b_qy),
        "bqg": bvec(b_qg),
        "watt": bvec(W_att),
        "mask": np.ascontiguousarray(mask),
        "ident": np.eye(128, dtype=np.float32),
    }

    def xt8(xs):
        # [BR, IN] -> [128, KC2, 2, BR] fp8: xt[p,k2,j,r] = S_X*x[r, 256k2+128j+p]
        t = np.clip(xs.T * S_X, -240, 240)  # [IN, BR]
        return np.ascontiguousarray(
            t.reshape(KC2L, 2, 128, BR).transpose(2, 0, 1, 3)
        ).astype(E4)

    per_core = []
    for c in range(NCORES):
        hs = hist[c * BL : (c + 1) * BL].reshape(BR, IN)
        qs = ques[c * BL : (c + 1) * BL].reshape(BR, IN)
        per_core.append(
            {
                "qt": xt8(qs),
                "ht": xt8(hs),
                "hn": np.ascontiguousarray(
                    hs.reshape(2, 128, IN).transpose(1, 0, 2)
                ).astype(ml_dtypes.bfloat16),
            }
        )
    return shared, per_core


# ---------------------------------------------------------------------------
# H-sharded DoubleRow program: each core computes ONE 128-dim h-chunk of the
# embeddings for the FULL batch (weight reuse -> fp8 DoubleRow at 2x rate,
# 1/8th the weight DMA), accumulates its num/den^2 partials for all examples,
# ReduceScatters them (sum over h-chunks, scatter by example owner), then
# runs the softmax+feat tail for its own 8 examples.
# ---------------------------------------------------------------------------

BRT = B * R          # 2048 rows, full batch
KC2 = IN // 256      # 8 double-row contraction chunks
NB = 4               # GEMM row blocks of 512
HB = 16              # 128-row blocks for num/den partials
FP8 = mybir.dt.float8e4


def build_program_hs(zero_bias=True):
    nc = bass.Bass(num_devices=NCORES)
    DR = mybir.MatmulPerfMode.DoubleRow
    descale = 1.0 / (S_X * S_W)

    qt_d = nc.dram_tensor("qt", [128, KC2, 2, BRT], FP8, kind="ExternalInput")
    ht_d = nc.dram_tensor("ht", [128, KC2, 2, BRT], FP8, kind="ExternalInput")
    w_d = nc.dram_tensor("wd", [128, 2, 2, KC2, 2, 128], FP8, kind="ExternalInput")
    hn_d = nc.dram_tensor("hn", [128, 2, IN], BF16, kind="ExternalInput")
    bias_d = nc.dram_tensor("bias4", [128, 4], F32, kind="ExternalInput")
    watt_d = nc.dram_tensor("wattc", [128, 1], F32, kind="ExternalInput")
    mask_d = nc.dram_tensor("maskr2", [128, 2, 32], F32, kind="ExternalInput")
    ident_d = nc.dram_tensor("ident", [128, 128], F32, kind="ExternalInput")
    feat_d = nc.dram_tensor("feat", [2, 128, IN], BF16, kind="ExternalOutput")

    ACT = mybir.ActivationFunctionType

    with tile.TileContext(nc) as tc:
        with (
            tc.tile_pool(name="big", bufs=1) as big,
            tc.tile_pool(name="tmp", bufs=3) as tmp,
            tc.tile_pool(name="sm", bufs=1) as sm,
            tc.tile_pool(name="dram", bufs=1, space="DRAM") as dram,
        ):
            # weights: tiny (1MB), load first
            w4 = big.tile([128, 2, 2, KC2, 2, 128], FP8, tag="w4")
            nc.sync.dma_start(w4[:], w_d[:])
            bias4 = sm.tile([128, 4], F32, tag="bias4")
            nc.sync.dma_start(bias4[:], bias_d[:])
            watt = sm.tile([128, 1], F32, tag="watt")
            nc.sync.dma_start(watt[:], watt_d[:])

            # full-batch transposed activations, chunked DMA for overlap
            qt = big.tile([128, KC2, 2, BRT], FP8, tag="qt")
            for k2 in range(KC2):
                nc.sync.dma_start(qt[:, k2], qt_d[:, k2])
            ht = big.tile([128, KC2, 2, BRT], FP8, tag="ht")
            for k2 in range(KC2):
                nc.sync.dma_start(ht[:, k2], ht_d[:, k2])

            # full-batch embedding products (this core's 128 h-dims)
            qew = big.tile([128, BRT], BF16, tag="qew")
            qe2 = big.tile([128, BRT], BF16, tag="qe2")
            he = big.tile([128, BRT], BF16, tag="he")
            he2 = big.tile([128, BRT], BF16, tag="he2")
            tyq = big.tile([128, BRT], BF16, tag="tyq")
            tyh = big.tile([128, BRT], BF16, tag="tyh")

            # num/den^2 partials, compacted to [p, hb, 64] (num | den2)
            ndsb = big.tile([128, HB, 64], BF16, tag="ndsb")

            cc_in = dram.tile([BRT, 64], BF16, tag="cc_in")
            cc_out = dram.tile([BRT // NCORES, 64], BF16, tag="cc_out")

            with (
                tc.tile_pool(name="psg", bufs=1, space="PSUM") as psg,
                tc.tile_pool(name="psnd", bufs=2, space="PSUM") as psnd,
            ):
                gbank = [
                    psg.tile([128, 512], F32, name=f"gb{rb}", tag=f"gb{rb}")
                    for rb in range(NB)
                ]

                def hb_numden(hb):
                    """num/den^2 partial for 128-row block hb + diag compaction."""
                    sl = slice(128 * hb, 128 * (hb + 1))
                    nps = psnd.tile([128, 128], F32, tag="nps")
                    nc.tensor.matmul(
                        nps[:], qew[:, sl], he[:, sl], start=True, stop=True
                    )
                    dps = psnd.tile([128, 128], F32, tag="dps")
                    nc.tensor.matmul(
                        dps[:], qe2[:, sl], he2[:, sl], start=True, stop=True
                    )
                    for e in range(4):
                        rsl = slice(32 * e, 32 * (e + 1))
                        nc.scalar.copy(ndsb[rsl, hb, 0:32], nps[rsl, rsl])
                        nc.vector.tensor_copy(
                            ndsb[rsl, hb, 32:64], dps[rsl, rsl]
                        )

                for i, xt in ((0, qt), (1, ht)):
                    for b in range(2):
                        for k2 in range(KC2):
                            for rb in range(NB):
                                nc.tensor.matmul(
                                    gbank[rb][:],
                                    w4[:, i, b, k2],
                                    xt[:, k2, :, 512 * rb : 512 * (rb + 1)],
                                    start=(k2 == 0),
                                    stop=(k2 == KC2 - 1),
                                    perf_mode=DR,
                                )
                        ty = (tyq, tyh)[i]
                        col = 2 * i + b
                        for rb in range(NB):
                            ps = gbank[rb]
                            dst = slice(512 * rb, 512 * (rb + 1))
                            if b == 0:
                                # y branch: ty = tanh(descale*ps + bias)
                                nc.scalar.activation(
                                    ty[:, dst], ps[:], ACT.Tanh,
                                    bias=bias4[:, col : col + 1],
                                    scale=descale,
                                )
                            else:
                                # g branch: tg = lrelu(ps); scale cancels
                                t1 = tmp.tile([128, 512], F32, tag="t1")
                                tg = tmp.tile([128, 512], F32, tag="tg")
                                if zero_bias:
                                    nc.vector.tensor_scalar_mul(
                                        t1[:], ps[:], 0.01
                                    )
                                    nc.vector.tensor_max(tg[:], ps[:], t1[:])
                                else:
                                    u = tmp.tile([128, 512], F32, tag="u")
                                    nc.scalar.activation(
                                        u[:], ps[:], ACT.Identity,
                                        bias=bias4[:, col : col + 1],
                                        scale=descale,
                                    )
                                    nc.vector.tensor_scalar_mul(
                                        t1[:], u[:], 0.01
                                    )
                                    nc.vector.tensor_max(tg[:], u[:], t1[:])
                                if i == 0:
                                    # qew = (ty*watt)*tg ; qe2 = (ty*tg)^2
                                    nc.vector.scalar_tensor_tensor(
                                        qew[:, dst], ty[:, dst], watt[:, 0:1],
                                        tg[:],
                                        op0=mybir.AluOpType.mult,
                                        op1=mybir.AluOpType.mult,
                                    )
                                    qe = tmp.tile([128, 512], F32, tag="qe")
                                    nc.vector.tensor_mul(qe[:], ty[:, dst], tg[:])
                                    nc.scalar.square(qe2[:, dst], qe[:])
                                else:
                                    nc.vector.tensor_mul(he[:, dst], ty[:, dst], tg[:])
                                    nc.scalar.square(he2[:, dst], he[:, dst])
                                    # num/den partials for finished rows
                                    for hb in range(4 * rb, 4 * rb + 4):
                                        hb_numden(hb)
                        if i == 0 and b == 0:
                            # consts land while GEMMs run
                            mask = sm.tile([128, 2, 32], F32, tag="mask")
                            nc.sync.dma_start(mask[:], mask_d[:])
                            ident = sm.tile([128, 128], F32, tag="ident")
                            nc.sync.dma_start(ident[:], ident_d[:])
                            hn = big.tile([128, 2, IN], BF16, tag="hn")
                            nc.sync.dma_start(hn[:], hn_d[:])

            # exchange partials: sum over h-chunks, scatter by example owner
            nc.sync.dma_start(
                cc_in[:].rearrange("(hb p) c -> p hb c", p=128), ndsb[:]
            )
            nc.gpsimd.collective_compute(
                "ReduceScatter",
                mybir.AluOpType.add,
                replica_groups=[list(range(NCORES))],
                ins=[cc_in.opt()],
                outs=[cc_out.opt()],
            )
            scnd = sm.tile([128, 2, 64], BF16, tag="scnd")
            nc.sync.dma_start(
                scnd[:], cc_out[:].rearrange("(g p) c -> p g c", p=128)
            )

            # attention tail for own 8 examples
            with (
                tc.tile_pool(name="psa", bufs=1, space="PSUM") as psa,
                tc.tile_pool(name="psf", bufs=4, space="PSUM") as psf,
            ):
                sd = tmp.tile([128, 2, 32], F32, tag="sd")
                nc.scalar.activation(sd[:], scnd[:, :, 32:64], ACT.Sqrt)
                rd = tmp.tile([128, 2, 32], F32, tag="rd")
                nc.vector.reciprocal(rd[:], sd[:])
                s = tmp.tile([128, 2, 32], F32, tag="s")
                nc.vector.tensor_mul(s[:], scnd[:, :, 0:32], rd[:])
                nc.vector.tensor_add(s[:], s[:], mask[:])
                att = sm.tile([128, 2, 32], F32, tag="att")
                nc.scalar.activation(att[:], s[:], ACT.Exp)
                rs = sm.tile([128, 2, 1], F32, tag="rs")
                nc.vector.reduce_sum(rs[:], att[:], axis=mybir.AxisListType.X)
                rrs = sm.tile([128, 2, 1], F32, tag="rrs")
                nc.vector.reciprocal(rrs[:], rs[:])
                for g in range(2):
                    attb = sm.tile([128, 128], F32, name=f"attb{g}", tag=f"attb{g}")
                    nc.vector.memset(attb[:], 0.0)
                    for e in range(4):
                        rsl = slice(32 * e, 32 * (e + 1))
                        nc.vector.tensor_scalar_mul(
                            attb[rsl, rsl], att[rsl, g, :], rrs[rsl, g, :]
                        )
                    atp = psa.tile([128, 128], F32, tag="atp")
                    nc.tensor.transpose(atp[:], attb[:], ident[:])
                    atb = sm.tile([128, 128], BF16, name=f"atb{g}", tag=f"atb{g}")
                    nc.scalar.copy(atb[:], atp[:])
                    for c2 in range(2):
                        fsb = tmp.tile([128, 1024], BF16, tag="fsb")
                        for half in range(2):
                            c = 2 * c2 + half
                            cs = slice(512 * c, 512 * (c + 1))
                            fps = psf.tile([128, 512], F32, tag="fps")
                            nc.tensor.matmul(
                                fps[:], atb[:], hn[:, g, cs], start=True, stop=True
                            )
                            dst = fsb[:, 512 * half : 512 * (half + 1)]
                            if half == 0:
                                nc.scalar.copy(dst, fps[:])
                            else:
                                nc.vector.tensor_copy(dst, fps[:])
                        nc.sync.dma_start(
                            feat_d[g, :, 1024 * c2 : 1024 * (c2 + 1)], fsb[:]
                        )

    _split_multi_waits(nc)
    return nc


def _prep_hs(hist, ques, W_hy, b_hy, W_hg, b_hg, W_qy, b_qy, W_qg, b_qg, W_att):
    """Host prep for the H-sharded program. Returns (shared over cores: qt, ht,
    mask, ident) and per-core (wd, bias4, wattc, hn)."""
    E4 = ml_dtypes.float8_e4m3

    def xt8(x):
        # [B,R,IN] -> X^T [IN, BRT] scaled fp8, packed [128, KC2, 2, BRT]
        xt = np.clip(x.reshape(BRT, IN).T * S_X, -240, 240)
        return np.ascontiguousarray(
            xt.reshape(KC2, 2, 128, BRT).transpose(2, 0, 1, 3)
        ).astype(E4)

    qt8 = xt8(ques)
    ht8 = xt8(hist)
    m32 = np.where(
        np.arange(32)[None, :] <= np.arange(32)[:, None], 0.0, NEG
    ).astype(np.float32)
    maskr = m32[np.tile(np.arange(32), 4)]  # [128, 32], row p -> round p%32
    maskr2 = np.ascontiguousarray(
        np.broadcast_to(maskr[:, None, :], (128, 2, 32))
    ).astype(np.float32)
    ident = np.eye(128, dtype=np.float32)

    Ws = [[W_qy, W_qg], [W_hy, W_hg]]
    bs = [b_qy, b_qg, b_hy, b_hg]
    per_core = []
    for c in range(NCORES):
        hsl = slice(128 * c, 128 * (c + 1))
        blocks = np.empty((2, 2, 128, KC2, 2, 128), dtype=E4)
        for i in range(2):
            for b in range(2):
                Wc = np.clip(Ws[i][b][:, hsl] * S_W, -240, 240)
                blocks[i, b] = (
                    Wc.reshape(KC2, 2, 128, 128).transpose(2, 0, 1, 3)
                ).astype(E4)
        wd = np.ascontiguousarray(blocks.transpose(2, 0, 1, 3, 4, 5))
        bias4 = np.ascontiguousarray(
            np.stack([bv[hsl] for bv in bs], axis=1)
        ).astype(np.float32)
        wattc = np.ascontiguousarray(W_att[hsl, None]).astype(np.float32)
        hs = hist[BL * c : BL * (c + 1)].reshape(BR, IN)
        hn = np.ascontiguousarray(
            hs.reshape(2, 128, IN).transpose(1, 0, 2)
        ).astype(ml_dtypes.bfloat16)
        per_core.append(
            {"wd": wd, "bias4": bias4, "wattc": wattc, "hn": hn}
        )
    shared = {"qt": qt8, "ht": ht8, "maskr2": maskr2, "ident": ident}
    return shared, per_core


# ---------------------------------------------------------------------------
# Weight-stationary data-parallel program ("ws8"): fp8 DoubleRow GEMMs with
# the WEIGHT tile stationary ([256k x 128h] per LDWEIGHTS) and the transposed
# activations moving ([256k x 256 rows], N=256). The GEMM output lands
# directly in [h-dim, rows] orientation -- no PE transposes and no per-tile
# evacuation storm. The leaky-relu branch runs on the Scalar engine via
# Prelu(alpha=0.01), which shares the exp_and_others activation table with
# Tanh/Square/Exp/Copy, so the only table switches are sqrt (preloaded early)
# and the switch back to exp (preloaded under the DVE score chain).
# num/den^2 accumulate in PSUM interleaved with the hist-side GEMMs.
# ---------------------------------------------------------------------------

KC2W = IN // 256  # 8 double-row contraction chunks
MCW = H // 128    # 8 h chunks per branch


def build_program_ws(zero_bias=True):
    # Bass.__init__ emits 4 gpsimd const memsets + an all-engine barrier;
    # the Pool/Q7 engine takes ~6.3us to boot, so that barrier holds EVERY
    # engine (incl. the DMA rings) idle until ~7us. We never read the const
    # APs (all activation biases are passed as real APs), so the barrier is
    # safe to elide.
    _orig_barrier = bass.Bass.all_engine_barrier
    bass.Bass.all_engine_barrier = lambda self, *, sem_only=False: None
    try:
        nc = bass.Bass()
    finally:
        bass.Bass.all_engine_barrier = _orig_barrier
    DR = mybir.MatmulPerfMode.DoubleRow
    descale = 1.0 / (S_X * S_W)

    qt_d = nc.dram_tensor("qt", [128, KC2W, 2, BR], FP8, kind="ExternalInput")
    ht_d = nc.dram_tensor("ht", [128, KC2W, 2, BR], FP8, kind="ExternalInput")
    # stationary weights: [128p, side, mg(=2m+branch), k2, j, 128 hcol]
    w_d = nc.dram_tensor(
        "wd", [128, 2, 2 * MCW, KC2W, 2, 128], FP8, kind="ExternalInput"
    )
    hn_d = nc.dram_tensor("hn", [128, 2, IN], BF16, kind="ExternalInput")
    # consts packed in one DMA: watt(8) | mask(128) | ident(128) | zeros(1)
    wmi_d = nc.dram_tensor("wmi", [128, MCW + 257], F32, kind="ExternalInput")
    feat_d = nc.dram_tensor("feat", [2, 128, IN], BF16, kind="ExternalOutput")

    ACT = mybir.ActivationFunctionType

    with tile.TileContext(nc) as tc:
        with (
            tc.tile_pool(name="big", bufs=1) as big,
            tc.tile_pool(name="tmp", bufs=3) as tmp,
            tc.tile_pool(name="sm", bufs=1) as sm,
        ):
            # DMA strategy: HWDGE descriptor generation caps each HW ring at
            # ~24ns/descriptor (one per partition per DMA), i.e. ~85GB/s at
            # 2KB lines. SWDGE (gpsimd) generates descriptors in software at
            # negligible cost, so the bulk stream (weights, activations,
            # consts) goes on the gpsimd queue in exact consumption order;
            # ht/hn ride the scalar HWDGE ring in parallel, feat-out uses
            # the idle sync ring at the end.
            # Hybrid DMA: the HWDGE rings (fast first-byte, ~90GB/s/ring
            # desc-gen cap) carry only the first-needed pieces; SWDGE
            # (slow serial Q7 emission ~2.8us/dma but wire-rate streaming)
            # carries the bulk weight stream in consumption order.
            w4 = big.tile([128, 2, 2 * MCW, KC2W, 2, 128], FP8, tag="w4")
            nc.sync.dma_start(w4[:, 0, 0:2], w_d[:, 0, 0:2])
            qt = big.tile([128, KC2W, 2, BR], FP8, tag="qt")
            nc.scalar.dma_start(qt[:], qt_d[:])
            ht = big.tile([128, KC2W, 2, BR], FP8, tag="ht")
            nc.scalar.dma_start(ht[:], ht_d[:])
            wmi = sm.tile([128, MCW + 257], F32, tag="wmi")
            nc.scalar.dma_start(wmi[:], wmi_d[:])
            watt = wmi[:, 0:MCW]
            mask = wmi[:, MCW : MCW + 128]
            ident = wmi[:, MCW + 128 : MCW + 256]
            # SWDGE stream, ordered by need-time (Q7 emits ~2.8us per dma);
            # consts ride the scalar ring so every SWDGE slot carries weights
            nc.gpsimd.dma_start(w4[:, 0, 2:4], w_d[:, 0, 2:4])
            nc.gpsimd.dma_start(w4[:, 0, 4:8], w_d[:, 0, 4:8])
            nc.gpsimd.dma_start(w4[:, 0, 8:12], w_d[:, 0, 8:12])
            nc.gpsimd.dma_start(w4[:, 0, 12:16], w_d[:, 0, 12:16])
            nc.gpsimd.dma_start(w4[:, 1, 0:4], w_d[:, 1, 0:4])
            nc.gpsimd.dma_start(w4[:, 1, 4:8], w_d[:, 1, 4:8])
            nc.gpsimd.dma_start(w4[:, 1, 8:12], w_d[:, 1, 8:12])
            nc.gpsimd.dma_start(w4[:, 1, 12:16], w_d[:, 1, 12:16])
            hn = big.tile([128, 2, IN], BF16, tag="hn")
            nc.gpsimd.dma_start(hn[:], hn_d[:])

            # warm the PE's DVFS ramp while the first weights stream in:
            # dependency-free matmuls on a scratch tile
            scr = big.tile([128, 2, 256], FP8, tag="scr")
            nc.vector.memset(scr[:], 0.0)

            qew = big.tile([128, MCW, BR], BF16, tag="qew")
            qe2 = big.tile([128, MCW, BR], BF16, tag="qe2")
            he = big.tile([128, MCW, BR], BF16, tag="he")
            he2 = big.tile([128, MCW, BR], BF16, tag="he2")

            assert zero_bias, "ws8 supports zero biases only"

            with (
                tc.tile_pool(name="pse", bufs=3, space="PSUM") as pse,
                tc.tile_pool(name="psnd", bufs=1, space="PSUM") as psnd,
            ):
                # num and den each pack both row-groups into one bank:
                # group 0's m==0 start=True wipes the whole bank, group 1
                # accumulates with start=False onto the zeroed half. The two
                # freed banks give the GEMM pipeline a third bank pair.
                ndA = psnd.tile([128, 512], F32, name="ndA", tag="ndA")
                ndB = psnd.tile([128, 512], F32, name="ndB", tag="ndB")

                # PE warmup during the initial DMA wait (borrows the nd0
                # bank; the num m=0 start=True wipe erases the garbage).
                # Sized to end just as the first weights land: an idle gap
                # right before the real matmuls re-triggers the ~3.4us DVFS
                # re-ramp at half rate.
                for _ in range(55):
                    nc.tensor.matmul(
                        ndA[:, 0:BR], scr[:, :, 0:128], scr[:],
                        start=True, stop=True, perf_mode=DR,
                        skip_group_check=True,
                    )
                last_tg = [None]

                def gemm_pair(i, m, xt):
                    """GEMMs for h-chunk m of side i: y and g accumulate in
                    two full banks, consecutive matmuls alternating banks
                    (same-bank back-to-back accumulation halves the PE
                    rate)."""
                    psy = pse.tile([128, 512], F32, tag="psy")
                    psg = pse.tile([128, 512], F32, tag="psg")
                    for k2 in range(KC2W):
                        for br, ps in ((0, psy), (1, psg)):
                            nc.tensor.matmul(
                                ps[:, 0:BR],
                                w4[:, i, 2 * m + br, k2],
                                xt[:, k2],
                                start=(k2 == 0),
                                stop=(k2 == KC2W - 1),
                                perf_mode=DR,
                            )
                    return psy, psg

                def evac(i, m, psy, psg):
                    """W_att is folded into the HIST side (per-partition in
                    this layout) so the ques evac needs no consts at all.
                    The final chunk's leaky-relu runs on DVE so the Scalar
                    engine is free to switch activation tables right after
                    its last tanh."""
                    ty = tmp.tile([128, BR], F32, tag="ty")
                    nc.scalar.activation(
                        ty[:], psy[:, 0:BR], ACT.Tanh, scale=descale
                    )
                    tg = tmp.tile([128, BR], F32, tag="tg")
                    nc.scalar.activation(tg[:], psg[:, 0:BR], ACT.Prelu, alpha=0.01)
                    last_tg[0] = tg
                    if i == 0:
                        nc.vector.tensor_mul(qew[:, m, :], ty[:], tg[:])
                        nc.vector.tensor_mul(
                            qe2[:, m, :], qew[:, m, :], qew[:, m, :]
                        )
                    else:
                        hetmp = tmp.tile([128, BR], F32, tag="hetmp")
                        nc.vector.tensor_mul(hetmp[:], ty[:], tg[:])
                        nc.vector.tensor_scalar_mul(
                            he[:, m, :], hetmp[:], watt[:, m : m + 1]
                        )
                        nc.vector.tensor_mul(he2[:, m, :], hetmp[:], hetmp[:])

                def numden(m, g):
                    sl = slice(128 * g, 128 * (g + 1))
                    dst = slice(128 * g, 128 * (g + 1))
                    nc.tensor.matmul(
                        ndA[:, dst], qew[:, m, sl], he[:, m, sl],
                        start=(m == 0 and g == 0), stop=(m == MCW - 1),
                        skip_group_check=(g == 1),
                    )
                    nc.tensor.matmul(
                        ndB[:, dst], qe2[:, m, sl], he2[:, m, sl],
                        start=(m == 0 and g == 0), stop=(m == MCW - 1),
                        skip_group_check=(g == 1),
                    )

                # ques side: embeddings chunk by chunk
                for m in range(MCW):
                    psy, psg = gemm_pair(0, m, qt)
                    evac(0, m, psy, psg)

                # hist side: GEMM chunk m, then group-0 num/den for chunk
                # m-1 so the PE never waits on the Scalar/DVE evac chain.
                # Group 1 runs as one burst at the end: its matmuls fill the
                # PE while group 0's softmax chain runs on Scalar/DVE.
                prev = None
                for m in range(MCW):
                    psy, psg = gemm_pair(1, m, ht)
                    if prev is not None:
                        numden(prev, 0)
                    evac(1, m, psy, psg)
                    prev = m
                numden(prev, 0)
                for m in range(MCW):
                    numden(m, 1)

                # rsqrt via ln+exp: both live in natural_log_exp_and_others,
                # so the tail needs only ONE table switch. The preload reads
                # the LAST prelu's output so the scheduler cannot hoist it
                # before the tanh/prelu phase (abs() first: ln(neg) is junk
                # but the result is discarded).
                dummy = tmp.tile([128, 1], F32, tag="dummy")
                nc.scalar.activation(dummy[:], last_tg[0][:, 0:1], ACT.Ln)

                sc = []
                for g in range(2):
                    gs = slice(128 * g, 128 * (g + 1))
                    sd = tmp.tile([128, 128], F32, tag="sd")
                    nc.scalar.activation(sd[:], ndB[:, gs], ACT.Ln)
                    rd = tmp.tile([128, 128], F32, tag="rd")
                    nc.scalar.activation(rd[:], sd[:], ACT.Exp, scale=-0.5)
                    s = sm.tile([128, 128], F32, name=f"sc{g}", tag=f"sc{g}")
                    nc.vector.tensor_mul(s[:], ndA[:, gs], rd[:])
                    nc.vector.tensor_add(s[:], s[:], mask[:])
                    sc.append(s)

            # attention tail + feat
            with (
                tc.tile_pool(name="psa", bufs=1, space="PSUM") as psa,
                tc.tile_pool(name="psf", bufs=4, space="PSUM") as psf,
            ):
                for g in range(2):
                    s = sc[g]
                    # unnormalized softmax: the 1/rowsum lands on the feat
                    # rows during PSUM evacuation (q is the output partition
                    # dim of the feat matmul)
                    att = sm.tile([128, 128], F32, name=f"att{g}", tag=f"att{g}")
                    rs = sm.tile([128, 1], F32, name=f"rs{g}", tag=f"rs{g}")
                    nc.scalar.activation(att[:], s[:], ACT.Exp, accum_out=rs[:])
                    rrs = sm.tile([128, 1], F32, name=f"rrs{g}", tag=f"rrs{g}")
                    nc.vector.reciprocal(rrs[:], rs[:])
                    atp = psa.tile([128, 128], F32, tag="atp")
                    nc.tensor.transpose(atp[:], att[:], ident[:])
                    atb = sm.tile([128, 128], BF16, name=f"atb{g}", tag=f"atb{g}")
                    nc.scalar.copy(atb[:], atp[:])
                    fsb = tmp.tile([128, IN], BF16, name=f"fsb{g}", tag=f"fsb{g}")
                    for c in range(4):
                        cs = slice(512 * c, 512 * (c + 1))
                        fps = psf.tile([128, 512], F32, tag="fps")
                        for h2 in range(2):
                            hs = slice(512 * c + 256 * h2, 512 * c + 256 * (h2 + 1))
                            nc.tensor.matmul(
                                fps[:, 256 * h2 : 256 * (h2 + 1)], atb[:],
                                hn[:, g, hs], start=(h2 == 0), stop=(h2 == 1),
                                skip_group_check=True,
                            )
                        if c % 2 == 0:
                            nc.scalar.mul(fsb[:, cs], fps[:], rrs[:, 0:1])
                        else:
                            nc.vector.tensor_scalar_mul(
                                fsb[:, cs], fps[:], rrs[:, 0:1]
                            )
                    eng = nc.sync if g == 0 else nc.scalar
                    eng.dma_start(feat_d[g], fsb[:])

    _split_multi_waits(nc)
    return nc


def _prep_ws(hist, ques, W_hy, b_hy, W_hg, b_hg, W_qy, b_qy, W_qg, b_qg, W_att):
    """Host prep for the weight-stationary program."""
    E4 = ml_dtypes.float8_e4m3

    # wd[p, i, 2m+b, k2, j, mcol] = S_W * W[i][b][256*k2+128*j+p, 128*m+mcol]
    wd = np.empty((128, 2, 2 * MCW, KC2W, 2, 128), dtype=E4)
    Ws = [[W_qy, W_qg], [W_hy, W_hg]]
    for i in range(2):
        for b in range(2):
            Wc = np.clip(Ws[i][b] * S_W, -240, 240).astype(E4)
            # [IN, H] -> [k2, j, p, m, mcol] -> [p, k2, j, m, mcol]
            Wr = Wc.reshape(KC2W, 2, 128, MCW, 128).transpose(2, 0, 1, 3, 4)
            wd[:, i, b::2] = Wr.transpose(0, 3, 1, 2, 4)

    wattw = np.ascontiguousarray(
        np.asarray(W_att, np.float32).reshape(MCW, 128).T
    ).astype(np.float32)

    m32 = np.where(
        np.arange(32)[None, :] <= np.arange(32)[:, None], 0.0, NEG
    ).astype(np.float32)
    mask = np.full((128, 128), NEG, np.float32)
    for e in range(4):
        mask[32 * e : 32 * e + 32, 32 * e : 32 * e + 32] = m32

    wmi = np.concatenate(
        [
            wattw, mask, np.eye(128, dtype=np.float32),
            np.zeros((128, 1), np.float32),
        ],
        axis=1,
    )
    shared = {
        "wd": np.ascontiguousarray(wd),
        "wmi": np.ascontiguousarray(wmi),
    }

    def xt8(xs):
        # [BR, IN] -> [128, KC2W, 2, BR]: xt[p,k2,j,r] = S_X*x[r, 256k2+128j+p]
        t = np.clip(xs.T * S_X, -240, 240)
        return np.ascontiguousarray(
            t.reshape(KC2W, 2, 128, BR).transpose(2, 0, 1, 3)
        ).astype(E4)

    per_core = []
    for c in range(NCORES):
        hs = hist[c * BL : (c + 1) * BL].reshape(BR, IN)
        qs = ques[c * BL : (c + 1) * BL].reshape(BR, IN)
        per_core.append(
            {
                "qt": xt8(qs),
                "ht": xt8(hs),
                "hn": np.ascontiguousarray(
                    hs.reshape(2, 128, IN).transpose(1, 0, 2)
                ).astype(ml_dtypes.bfloat16),
            }
        )
    return shared, per_core


# ---------------------------------------------------------------------------
# Host side
# ---------------------------------------------------------------------------

_PROG_CACHE = {}


def _get_prog(mode, zero_bias):
    key = (mode, zero_bias)
    if key not in _PROG_CACHE:
        _PROG_CACHE[key] = build_program(mode, zero_bias)
    return _PROG_CACHE[key]


def _prep_shared(W_hy, b_hy, W_hg, b_hg, W_qy, b_qy, W_qg, b_qg, W_att, mode):
    xnp = _XNP[mode]
    ws = S_W if mode == "fp8" else 1.0

    def reblock(W):
        # [IN, H] -> [MC, 128, KC, 128]; Wr[m, p, k, h] = W[128k+p, 128m+h]
        Wv = np.clip(W * ws, -240, 240) if mode == "fp8" else W
        return np.ascontiguousarray(
            Wv.reshape(KC, 128, MC, 128).transpose(2, 1, 0, 3)
        ).astype(xnp)

    def bvec(b):
        return np.ascontiguousarray(b.reshape(MC, 128).T).astype(np.float32)

    m32 = np.where(
        np.arange(32)[None, :] <= np.arange(32)[:, None], 0.0, NEG
    ).astype(np.float32)
    # diag blocks causal, off-diag (cross-example) blocks fully masked
    mask = np.full((128, 128), NEG, np.float32)
    for e in range(4):
        mask[32 * e : 32 * e + 32, 32 * e : 32 * e + 32] = m32
    # [MC, 128, 2, KC, 128]: per-partition contiguous weight lines
    wh = np.ascontiguousarray(np.stack([reblock(W_hy), reblock(W_hg)], axis=2))
    wq = np.ascontiguousarray(np.stack([reblock(W_qy), reblock(W_qg)], axis=2))
    shared = {
        "wh": wh,
        "wq": wq,
        "bhy": bvec(b_hy),
        "bhg": bvec(b_hg),
        "bqy": bvec(b_qy),
        "bqg": bvec(b_qg),
        "watt": bvec(W_att),
        "mask": np.ascontiguousarray(mask),
        "ident": np.eye(128, dtype=np.float32),
    }
    return shared, xnp


def kernel(
    hist, ques, W_hy, b_hy, W_hg, b_hg, W_qy, b_qy, W_qg, b_qg, W_att, b_att,
    mode="ws8", trace=False,
):
    from concourse.bass_utils import run_bass_kernel_spmd

    hist = np.asarray(hist, np.float32)
    ques = np.asarray(ques, np.float32)
    zero_bias = all(
        not np.any(np.asarray(b)) for b in (b_hy, b_hg, b_qy, b_qg)
    )
    if mode in ("dp8dr", "ws8") and not zero_bias:
        mode = "fp8"
    if mode == "ws8":
        key = ("ws8", True)
        if key not in _PROG_CACHE:
            _PROG_CACHE[key] = build_program_ws(True)
        nc = _PROG_CACHE[key]
        shared, per_core = _prep_ws(
            hist, ques,
            np.asarray(W_hy, np.float32), np.asarray(b_hy, np.float32),
            np.asarray(W_hg, np.float32), np.asarray(b_hg, np.float32),
            np.asarray(W_qy, np.float32), np.asarray(b_qy, np.float32),
            np.asarray(W_qg, np.float32), np.asarray(b_qg, np.float32),
            np.asarray(W_att, np.float32),
        )
        in_maps = [{**shared, **pc} for pc in per_core]
        res = run_bass_kernel_spmd(
            nc, in_maps, core_ids=list(range(NCORES)), trace=trace
        )
        feat = np.concatenate(
            [
                r["feat"].reshape(BL, R, IN).astype(np.float32)
                for r in res.results
            ],
            axis=0,
        )
        if trace:
            return feat, res
        return feat
    if mode == "dp8dr":
        key = ("dp8dr", True)
        if key not in _PROG_CACHE:
            _PROG_CACHE[key] = build_program_dp(True)
        nc = _PROG_CACHE[key]
        shared, per_core = _prep_dp(
            hist, ques,
            np.asarray(W_hy, np.float32), np.asarray(b_hy, np.float32),
            np.asarray(W_hg, np.float32), np.asarray(b_hg, np.float32),
            np.asarray(W_qy, np.float32), np.asarray(b_qy, np.float32),
            np.asarray(W_qg, np.float32), np.asarray(b_qg, np.float32),
            np.asarray(W_att, np.float32),
        )
        in_maps = [{**shared, **pc} for pc in per_core]
        res = run_bass_kernel_spmd(
            nc, in_maps, core_ids=list(range(NCORES)), trace=trace
        )
        feat = np.concatenate(
            [
                r["feat"].reshape(BL, R, IN).astype(np.float32)
                for r in res.results
            ],
            axis=0,
        )
        if trace:
            return feat, res
        return feat
    if mode == "hs8":
        key = ("hs8", zero_bias)
        if key not in _PROG_CACHE:
            _PROG_CACHE[key] = build_program_hs(zero_bias)
        nc = _PROG_CACHE[key]
        shared, per_core = _prep_hs(
            hist, ques,
            np.asarray(W_hy, np.float32), np.asarray(b_hy, np.float32),
            np.asarray(W_hg, np.float32), np.asarray(b_hg, np.float32),
            np.asarray(W_qy, np.float32), np.asarray(b_qy, np.float32),
            np.asarray(W_qg, np.float32), np.asarray(b_qg, np.float32),
            np.asarray(W_att, np.float32),
        )
        in_maps = [{**shared, **pc} for pc in per_core]
        res = run_bass_kernel_spmd(
            nc, in_maps, core_ids=list(range(NCORES)), trace=trace
        )
        feat = np.concatenate(
            [
                r["feat"].reshape(BL, R, IN).astype(np.float32)
                for r in res.results
            ],
            axis=0,
        )
        if trace:
            return feat, res
        return feat
    nc = _get_prog(mode, zero_bias)
    shared, xnp = _prep_shared(
        np.asarray(W_hy, np.float32), np.asarray(b_hy, np.float32),
        np.asarray(W_hg, np.float32), np.asarray(b_hg, np.float32),
        np.asarray(W_qy, np.float32), np.asarray(b_qy, np.float32),
        np.asarray(W_qg, np.float32), np.asarray(b_qg, np.float32),
        np.asarray(W_att, np.float32), mode,
    )
    xs = S_X if mode == "fp8" else 1.0
    in_maps = []
    for c in range(NCORES):
        hs = hist[c * BL : (c + 1) * BL].reshape(BR, IN)
        qs = ques[c * BL : (c + 1) * BL].reshape(BR, IN)
        im = dict(shared)
        # [128, KC, BR]: partition-major transposed activations
        qsv = np.clip(qs * xs, -240, 240) if mode == "fp8" else qs
        hsv = np.clip(hs * xs, -240, 240) if mode == "fp8" else hs
        im["qt"] = np.ascontiguousarray(
            qsv.T.reshape(KC, 128, BR).transpose(1, 0, 2)
        ).astype(xnp)
        im["ht"] = np.ascontiguousarray(
            hsv.T.reshape(KC, 128, BR).transpose(1, 0, 2)
        ).astype(xnp)
        im["hn"] = np.ascontiguousarray(
            hs.reshape(2, 128, IN).transpose(1, 0, 2)
        ).astype(ml_dtypes.bfloat16)
        in_maps.append(im)

    res = run_bass_kernel_spmd(
        nc, in_maps, core_ids=list(range(NCORES)), trace=trace
    )
    feat = np.concatenate(
        [
            r["feat"].reshape(BL, R, IN).astype(np.float32)
            for r in res.results
        ],
        axis=0,
    )
    if trace:
        return feat, res
    return feat

